# revision 13
# baseline (speedup 1.0000x reference)
"""Trainium2 Bass kernel for fused LayerNorm + multi-head ALiBi attention.

Reference computation (B=2, S=2048, D=1024, H=16 heads, dh=64):
    xn = LayerNorm(x) * gamma + beta
    q,k,v = split_heads(xn @ Wq), ... ; att = softmax(q k^T / 8 + alibi); out = (att v) @ Wo

Sharding: 8 cores = 2 batches x 4 head-groups (4 heads each).  Each core
computes a partial output (its heads' contribution through its Wo row-slice);
host sums the 4 partials per batch (the tensor-parallel all-reduce).

Per-core layout strategy ("transposed" attention):
  - xn is transposed on-chip to xnT [D, S] via PE-transpose; gamma/beta are
    applied during the PSUM->SBUF evacuation (per-partition scalars there).
  - Q,K projections produce Q^T/K^T [head_dim, S]; scores are computed
    directly as scores^T[j, i] tiles (j on partitions), which is exactly the
    layout the PV matmul wants as its moving operand -- no p-transposes.
  - ALiBi (-c_h|i-j|) is folded into the score matmul via 2 extra contraction
    rows: k' = [k, j, 1], q'_lower = [q, 8c, -8c*i], q'_upper = -aug.  Tiles
    crossing the diagonal are computed with both variants and combined with an
    elementwise min (since -|x| = min(x, -x)).
  - Softmax skips max-subtraction (alibi <= 0 and qk/8 is O(1), so exp never
    overflows); the row-sums come for free from an appended ones-column in V
    (PV psum row 64).  Normalization divides PV psum rows by row 64; any
    per-column factor on p~ cancels in that ratio.
  - Matmul operands are bitcast to float32r (FP22): full PE rate at N>=256.
    p~ and V are stored bf16 (errors average out in the PV contraction).
"""

import numpy as np

import concourse.bass as bass
import concourse.tile as tile
from concourse import bacc, mybir
from concourse.bass_utils import run_bass_kernel_spmd
from concourse.masks import make_identity

F32 = mybir.dt.float32
F32R = mybir.dt.float32r
BF16 = mybir.dt.bfloat16
AF = mybir.ActivationFunctionType
OP = mybir.AluOpType

S = 2048
D = 1024
HD = 64          # head dim
NH = 4           # heads per core
INNER = NH * HD  # 256
P = 128
NTS = S // P     # 16 s-tiles
NDT = D // P     # 8 d-tiles
SI = 512         # i-tile width
NI = S // SI     # 4 i-tiles
NJT = S // P     # 16 j-tiles
KAUG = HD + 2    # 66 = augmented contraction for scores

_CACHE = {}


def _r(ap):
    return ap.bitcast(F32R)


def _build():
    nc = bacc.Bacc("TRN2", target_bir_lowering=False, debug=False, num_devices=8)

    xb = nc.dram_tensor("xb", [S, D], F32, kind="ExternalInput").ap()
    wq = nc.dram_tensor("wq", [D, INNER], F32, kind="ExternalInput").ap()
    wk = nc.dram_tensor("wk", [D, INNER], F32, kind="ExternalInput").ap()
    wv = nc.dram_tensor("wv", [D, INNER], F32, kind="ExternalInput").ap()
    wo = nc.dram_tensor("wo", [INNER, D], F32, kind="ExternalInput").ap()
    g8 = nc.dram_tensor("g8", [D], F32, kind="ExternalInput").ap()
    b8 = nc.dram_tensor("b8", [D], F32, kind="ExternalInput").ap()
    kaug_in = nc.dram_tensor("kaug", [2 * NH, S], F32, kind="ExternalInput").ap()
    qaugL_in = nc.dram_tensor("qaugL", [2 * NH, S], F32, kind="ExternalInput").ap()
    qaugU_in = nc.dram_tensor("qaugU", [2 * NH, S], F32, kind="ExternalInput").ap()
    dcor_in = nc.dram_tensor("dcor", [P, 1280], F32, kind="ExternalInput").ap()
    chn_in = nc.dram_tensor("chn", [P, NH], F32, kind="ExternalInput").ap()
    out_d = nc.dram_tensor("out", [S, D], F32, kind="ExternalOutput").ap()

    with tile.TileContext(nc) as tc:
        with (
            tc.tile_pool(name="consts", bufs=1) as consts,
            tc.tile_pool(name="wpool", bufs=1) as wpool,
            tc.tile_pool(name="xnt_pool", bufs=1) as xnt_pool,
            tc.tile_pool(name="qk", bufs=1) as qkpool,
            tc.tile_pool(name="vpool", bufs=1) as vpool,
            tc.tile_pool(name="otpool", bufs=1) as otpool,
            tc.tile_pool(name="xio", bufs=2) as xio,
            tc.tile_pool(name="xnp", bufs=2) as xnp,
            tc.tile_pool(name="small", bufs=4) as small,
            tc.tile_pool(name="rcp", bufs=1) as rcp,
            tc.tile_pool(name="ptiles", bufs=7) as ptiles,
            tc.tile_pool(name="scp", bufs=2) as scp,
            tc.tile_pool(name="bcp", bufs=2) as bcp,
            tc.tile_pool(name="psA", bufs=4, space="PSUM") as psA,
            tc.tile_pool(name="psPV", bufs=2, space="PSUM") as psPV,
        ):
            ident = consts.tile([P, P], F32)
            make_identity(nc, ident)
            eps_t = consts.tile([P, 1], F32)
            nc.vector.memset(eps_t, 1e-5)
            gam = consts.tile([P, NDT], F32)
            bet = consts.tile([P, NDT], F32)
            nc.sync.dma_start(out=gam, in_=g8.rearrange("(t p) -> p t", p=P))
            nc.sync.dma_start(out=bet, in_=b8.rearrange("(t p) -> p t", p=P))

            wq_sb = wpool.tile([P, NDT, INNER], F32, tag="wq")
            wk_sb = wpool.tile([P, NDT, INNER], F32, tag="wk")
            wv_sb = wpool.tile([P, NDT, INNER], F32, tag="wv")
            wo_sb = wpool.tile([P, 2, D], F32, tag="wo")
            nc.sync.dma_start(out=_r(wq_sb), in_=_r(wq.rearrange("(t p) n -> p t n", p=P)))
            nc.sync.dma_start(out=_r(wk_sb), in_=_r(wk.rearrange("(t p) n -> p t n", p=P)))
            nc.sync.dma_start(out=_r(wv_sb), in_=_r(wv.rearrange("(t p) n -> p t n", p=P)))
            nc.sync.dma_start(out=_r(wo_sb), in_=_r(wo.rearrange("(t p) n -> p t n", p=P)))

            dcor = consts.tile([P, 1280], F32)
            chn = consts.tile([P, NH], F32)
            nc.sync.dma_start(out=chn, in_=chn_in)
            nc.sync.dma_start(out=dcor, in_=dcor_in)
            xnt = xnt_pool.tile([P, NDT, S], F32)

            # ---------------- LayerNorm + transpose to xnT ----------------
            for st in range(NTS):
                s0 = st * P
                x_t = xio.tile([P, D], F32, tag="xio")
                nc.sync.dma_start(out=x_t, in_=xb[s0 : s0 + P, :])

                stats = small.tile([P, 2, nc.vector.BN_STATS_DIM], F32, tag="stats")
                xr = x_t.rearrange("p (c f) -> p c f", c=2)
                for c in range(2):
                    nc.vector.bn_stats(out=stats[:, c, :], in_=xr[:, c, :])
                mv = small.tile([P, nc.vector.BN_AGGR_DIM], F32, tag="mv")
                nc.vector.bn_aggr(out=mv, in_=stats)
                # rinv = 1/sqrt(var+eps) = exp(-0.5*ln(var+eps)): stays in the
                # natural_log_exp ACT table set shared with the softmax exps.
                lnv = small.tile([P, 1], F32, tag="lnv")
                nc.scalar.activation(out=lnv, in_=mv[:, 1:2], func=AF.Ln, bias=eps_t)
                rinv = small.tile([P, 1], F32, tag="rinv")
                nc.scalar.activation(out=rinv, in_=lnv, func=AF.Exp, scale=-0.5)
                xn_t = xnp.tile([P, D], F32, tag="xn")
                nc.vector.tensor_scalar(
                    out=xn_t, in0=x_t, scalar1=mv[:, 0:1], scalar2=rinv,
                    op0=OP.subtract, op1=OP.mult,
                )
                for half in range(2):
                    pst = psA.tile([P, SI], F32, tag="psA")
                    for q in range(4):
                        dt = half * 4 + q
                        nc.tensor.transpose(
                            pst[:, q * P : (q + 1) * P],
                            xn_t[:, dt * P : (dt + 1) * P],
                            ident,
                        )
                    for q in range(4):
                        dt = half * 4 + q
                        nc.vector.tensor_scalar(
                            out=_r(xnt[:, dt, s0 : s0 + P]),
                            in0=pst[:, q * P : (q + 1) * P],
                            scalar1=gam[:, dt : dt + 1],
                            scalar2=bet[:, dt : dt + 1],
                            op0=OP.mult,
                            op1=OP.add,
                        )

            # ---------------- V projection (+ ones col) -> vaug bf16 -------
            vaug = vpool.tile([P, NTS, NH * (HD + 1)], BF16)
            va4 = vaug.rearrange("p t (h c) -> p t h c", h=NH)
            nc.vector.memset(va4[:, :, :, HD : HD + 1], 1.0)
            for st in range(NTS):
                psv = psA.tile([P, SI], F32, tag="psA")
                for kt in range(NDT):
                    nc.tensor.matmul(
                        psv[:, :INNER],
                        _r(xnt[:, kt, st * P : (st + 1) * P]),
                        _r(wv_sb[:, kt, :]),
                        start=(kt == 0),
                        stop=(kt == NDT - 1),
                    )
                nc.vector.tensor_copy(
                    out=va4[:, st, :, 0:HD],
                    in_=psv[:, :INNER].rearrange("p (h c) -> p h c", h=NH),
                )

            # ------------- per head-pair: QK projection + attention --------
            outT = otpool.tile([P, 2, S], F32)
            for pair in range(2):
                hA, hB = 2 * pair, 2 * pair + 1
                kg = {}
                qL = {}
                qU = {}
                for h in (hA, hB):
                    kg[h] = qkpool.tile([KAUG, S], F32, tag=f"kg{h % 2}", name=f"kg{h}")
                    qL[h] = qkpool.tile([KAUG, S], F32, tag=f"qL{h % 2}", name=f"qL{h}")
                    qU[h] = qkpool.tile([KAUG, S], F32, tag=f"qU{h % 2}", name=f"qU{h}")
                    nc.sync.dma_start(out=_r(kg[h][HD:KAUG, :]), in_=_r(kaug_in[2 * h : 2 * h + 2, :]))
                    nc.sync.dma_start(
                        out=_r(qL[h][HD:KAUG, :]), in_=_r(qaugL_in[2 * h : 2 * h + 2, :])
                    )
                    nc.sync.dma_start(
                        out=_r(qU[h][HD:KAUG, :]), in_=_r(qaugU_in[2 * h : 2 * h + 2, :])
                    )
                for i in range(NI):
                    i0 = i * SI
                    psq = psA.tile([P, SI], F32, tag="psA")
                    psk = psA.tile([P, SI], F32, tag="psA")
                    for kt in range(NDT):
                        nc.tensor.matmul(
                            psq,
                            _r(wq_sb[:, kt, pair * P : (pair + 1) * P]),
                            _r(xnt[:, kt, i0 : i0 + SI]),
                            start=(kt == 0),
                            stop=(kt == NDT - 1),
                        )
                    for kt in range(NDT):
                        nc.tensor.matmul(
                            psk,
                            _r(wk_sb[:, kt, pair * P : (pair + 1) * P]),
                            _r(xnt[:, kt, i0 : i0 + SI]),
                            start=(kt == 0),
                            stop=(kt == NDT - 1),
                        )
                    for h, lo in ((hA, 0), (hB, HD)):
                        src_q = psq[lo : lo + HD, :]
                        src_k = psk[lo : lo + HD, :]
                        nc.vector.tensor_copy(out=_r(qL[h][0:HD, i0 : i0 + SI]), in_=src_q)
                        nc.vector.tensor_copy(out=_r(qU[h][0:HD, i0 : i0 + SI]), in_=src_q)
                        nc.vector.tensor_copy(out=_r(kg[h][0:HD, i0 : i0 + SI]), in_=src_k)

                for h in (hA, hB):
                    for i in range(NI):
                        i0 = i * SI
                        pts = []
                        for jt in range(NJT):
                            j0 = jt * P
                            ps = psA.tile([P, SI], F32, tag="psA")
                            if j0 < i0:
                                nc.tensor.matmul(
                                    ps, _r(kg[h][:, j0 : j0 + P]),
                                    _r(qL[h][:, i0 : i0 + SI]),
                                )
                            elif j0 >= i0 + SI:
                                nc.tensor.matmul(
                                    ps, _r(kg[h][:, j0 : j0 + P]),
                                    _r(qU[h][:, i0 : i0 + SI]),
                                )
                            else:
                                k = (j0 - i0) // P
                                w = (k + 1) * P
                                off = (k * (k + 1) // 2) * P
                                nc.tensor.matmul(
                                    ps, _r(kg[h][:, j0 : j0 + P]),
                                    _r(qL[h][:, i0 : i0 + SI]),
                                )
                                sc = scp.tile([P, SI], F32, tag="sc")
                                nc.vector.scalar_tensor_tensor(
                                    out=sc[:, :w], in0=dcor[:, off : off + w],
                                    scalar=chn[:, h : h + 1], in1=ps[:, :w],
                                    op0=OP.mult, op1=OP.add,
                                )
                                if w < SI:
                                    nc.vector.tensor_copy(out=sc[:, w:], in_=ps[:, w:])
                                pt = ptiles.tile([P, SI], BF16, tag="pt")
                                nc.scalar.activation(
                                    out=pt, in_=sc, func=AF.Exp, scale=0.125
                                )
                                pts.append(pt)
                                continue
                            pt = ptiles.tile([P, SI], BF16, tag="pt")
                            nc.scalar.activation(
                                out=pt, in_=ps, func=AF.Exp, scale=0.125
                            )
                            pts.append(pt)
                        pso = psPV.tile([HD + 1, SI], F32, tag="pv")
                        for jt in range(NJT):
                            nc.tensor.matmul(
                                pso,
                                vaug[:, jt, h * (HD + 1) : (h + 1) * (HD + 1)],
                                pts[jt],
                                start=(jt == 0),
                                stop=(jt == NJT - 1),
                            )
                        rc = rcp.tile([1, SI], F32, tag="rc")
                        nc.vector.reciprocal(out=rc, in_=pso[HD : HD + 1, :])
                        bc = bcp.tile([HD, SI], F32, tag="bc")
                        nc.gpsimd.partition_broadcast(bc, rc)
                        nc.vector.tensor_tensor(
                            out=_r(outT[(h % 2) * HD : (h % 2) * HD + HD, pair, i0 : i0 + SI]),
                            in0=pso[0:HD, :],
                            in1=bc,
                            op=OP.mult,
                        )

            # ---------------- final projection F = out @ Wo ----------------
            for st in range(NTS):
                s0 = st * P
                f_t = xio.tile([P, D], F32, tag="xio")
                for n in range(2):
                    psf = psA.tile([P, SI], F32, tag="psA")
                    for t in range(2):
                        nc.tensor.matmul(
                            psf,
                            _r(outT[:, t, s0 : s0 + P]),
                            _r(wo_sb[:, t, n * SI : (n + 1) * SI]),
                            start=(t == 0),
                            stop=(t == 1),
                        )
                    nc.vector.tensor_copy(out=f_t[:, n * SI : (n + 1) * SI], in_=psf)
                nc.sync.dma_start(out=out_d[s0 : s0 + P, :], in_=f_t)

    nc.compile()
    return nc


def _core_inputs(x, ln_gamma, ln_beta, Wq, Wk, Wv, Wo):
    """Build the 8 per-core input maps."""
    iota = np.arange(S, dtype=np.float64)
    dcor = np.zeros((P, 1280), dtype=np.float32)
    pp = np.arange(P)
    for k in range(4):
        delta0, w, off = k * P, (k + 1) * P, (k * (k + 1) // 2) * P
        ff = np.arange(w)
        dcor[:, off : off + w] = 16.0 * np.maximum(
            delta0 + pp[:, None] - ff[None, :], 0
        )
    maps = []
    for c in range(8):
        b, hg = c // 4, c % 4
        cols = slice(hg * INNER, (hg + 1) * INNER)
        qaugL = np.zeros((2 * NH, S), dtype=np.float64)
        kaug = np.zeros((2 * NH, S), dtype=np.float64)
        chn = np.zeros((P, NH), dtype=np.float32)
        for h in range(NH):
            hgl = hg * NH + h
            ch = 2.0 ** (-8.0 / (16 - hgl))
            kaug[2 * h, :] = iota
            kaug[2 * h + 1, :] = 8.0 * ch
            qaugL[2 * h, :] = 8.0 * ch
            qaugL[2 * h + 1, :] = -iota
            chn[:, h] = -ch
        maps.append(
            {
                "xb": np.ascontiguousarray(x[b]),
                "wq": np.ascontiguousarray(Wq[:, cols]),
                "wk": np.ascontiguousarray(Wk[:, cols]),
                "wv": np.ascontiguousarray(Wv[:, cols]),
                "wo": np.ascontiguousarray(Wo[cols, :]),
                "g8": np.ascontiguousarray(ln_gamma),
                "b8": np.ascontiguousarray(ln_beta),
                "kaug": kaug.astype(np.float32),
                "qaugL": qaugL.astype(np.float32),
                "qaugU": (-qaugL).astype(np.float32),
                "dcor": dcor,
                "chn": chn,
            }
        )
    return maps


def kernel(x, ln_gamma, ln_beta, Wq, Wk, Wv, Wo, _trace=False):
    x = np.asarray(x, dtype=np.float32)
    if "nc" not in _CACHE:
        _CACHE["nc"] = _build()
    nc = _CACHE["nc"]
    maps = _core_inputs(
        x,
        np.asarray(ln_gamma, np.float32),
        np.asarray(ln_beta, np.float32),
        np.asarray(Wq, np.float32),
        np.asarray(Wk, np.float32),
        np.asarray(Wv, np.float32),
        np.asarray(Wo, np.float32),
    )
    res = run_bass_kernel_spmd(nc, maps, core_ids=list(range(8)), trace=_trace)
    parts = [res.results[c]["out"] for c in range(8)]
    out = np.stack(
        [
            parts[0] + parts[1] + parts[2] + parts[3],
            parts[4] + parts[5] + parts[6] + parts[7],
        ]
    )
    if _trace:
        _CACHE["last_result"] = res
    return out
